# revision 4
# baseline (speedup 1.0000x reference)
"""GQA attention kernel for Trainium2, sharded over 8 NeuronCores.

Sharding: tensor-parallel over heads. Core c owns kv-head c and q-heads
4c..4c+3 (rows 256c:256c+256 of Wq, rows 64c:64c+64 of Wk/Wv) and columns
256c:256c+256 of Wo. Each core computes a full-shape partial of the output
(o_proj column-parallel); the host sums the 8 partials (the all-reduce)
and adds bo.

v4: bf16 data path; context accumulated in [q, d] orientation with the
exp tiles as stationary operand (65-column streams); ctxT built by DMA
transpose; PE kept saturated by interleaving QKV-projection and o_proj
"fill units" into the ACT-bound softmax stretches. PSUM: scores 4 banks
+ ctx 2 banks + shared qkv/o_proj pool 2 banks.
"""

import os
import sys

for _p in ("/opt/trn_rl_repo",):
    if _p not in sys.path and os.path.isdir(_p):
        sys.path.insert(0, _p)

import ml_dtypes
import numpy as np

import concourse.bass as bass
import concourse.bacc as bacc
import concourse.tile as tile
from concourse import mybir
from concourse import bass_utils

F32 = mybir.dt.float32
BF16 = mybir.dt.bfloat16
AF = mybir.ActivationFunctionType
NPBF16 = ml_dtypes.bfloat16

B = 2
S = 2048
H = 2048
D = 64
N_CORES = 8
QH_PER_CORE = 4          # q-heads per core
QF = QH_PER_CORE * D     # 256 q features per core
TOK = B * S              # 4096
SCALE = 1.0 / np.sqrt(D)  # 0.125
CK = 512                 # token chunk for QKV projection
NCK = TOK // CK          # 8 chunks (4 per batch)

_CACHE = {}


def _build_program():
    nc = bacc.Bacc("TRN2", target_bir_lowering=False, debug=False)

    hsT = nc.dram_tensor("hsT", [H, TOK], BF16, kind="ExternalInput").ap()
    wqkvT = nc.dram_tensor("wqkvT", [H, 384], BF16, kind="ExternalInput").ap()
    woT = nc.dram_tensor("woT", [QF, H], BF16, kind="ExternalInput").ap()
    bqkv = nc.dram_tensor("bqkv", [128, 3], F32, kind="ExternalInput").ap()
    maskp = nc.dram_tensor("maskp", [128, B, S // 128], F32, kind="ExternalInput").ap()
    out = nc.dram_tensor("out", [B, S, H], BF16, kind="ExternalOutput").ap()

    with tile.TileContext(nc) as tc:
        with tc.tile_pool(name="const", bufs=1) as cp:
            bqkv_sb = cp.tile([128, 3], F32)
            nc.sync.dma_start(out=bqkv_sb, in_=bqkv)
            mask_sb = cp.tile([128, B, S // 128], F32)
            nc.sync.dma_start(out=mask_sb, in_=maskp)
            # K/V weight slice first: it gates the first projection unit.
            w_qkv = cp.tile([128, 16, 384], BF16)     # (p, h_tile, feature)
            wqkvT_t = wqkvT.rearrange("(t p) f -> p t f", p=128)
            nc.sync.dma_start(out=w_qkv[:, :, 256:384], in_=wqkvT_t[:, :, 256:384])
            w_o = cp.tile([128, 2, H], BF16)          # (p, f_tile, e)

            # Warm consumer engine vector clocks on the const DMAs so real
            # instructions keep to their single sync-wait slot.
            scratch = cp.tile([128, 1], F32)
            nc.scalar.copy(out=scratch, in_=mask_sb[:, 0, 0:1])
            scratch2 = cp.tile([128, 1], F32)
            nc.vector.tensor_copy(out=scratch2, in_=bqkv_sb[:, 0:1])

            # Q^T, K^T, V^T resident in SBUF: qkvT[0] = q feats 0:128,
            # qkvT[1] = q feats 128:256, qkvT[2] = [K (0:64) | V (64:128)].
            qkvT = [cp.tile([128, TOK], BF16, name=f"qkvT{i}") for i in range(3)]
            # Per-batch tiles (separate tensors so batch-1 setup writes carry
            # no false WAR against batch-0 attention reads).
            voness = [cp.tile([128, 16, 64], BF16, name=f"vones{b_}")
                      for b_ in range(B)]           # V rows (dense)
            ones_sb = cp.tile([128, 1], BF16)
            nc.vector.memset(ones_sb, 1.0)
            # K^T replicated in both partition halves (PE alignment rule).
            k2s = [cp.tile([128, S], BF16, name=f"k2_{b_}") for b_ in range(B)]

            hsT_tiled = hsT.rearrange("(t p) n -> p t n", p=128)

            with tc.tile_pool(name="hs_sb", bufs=4) as hsb, \
                 tc.tile_pool(name="att_sb", bufs=4) as asb, \
                 tc.tile_pool(name="osb_sb", bufs=4) as osb_pool, \
                 tc.tile_pool(name="drain_sb", bufs=8) as dsb, \
                 tc.tile_pool(name="cnorm_sb", bufs=3) as nsb, \
                 tc.tile_pool(name="ctxT_sb", bufs=2) as csb, \
                 tc.tile_pool(name="gemm_ps", bufs=2, space="PSUM") as gps:

                # scores/ctx psum pools closed before the epilogue so the
                # final o_proj can use a deeper psum ring (gemm2).
                _sps_cm = tc.tile_pool(name="scores_ps", bufs=2, space="PSUM")
                _xps_cm = tc.tile_pool(name="ctx_ps", bufs=2, space="PSUM")
                sps = _sps_cm.__enter__()
                xps = _xps_cm.__enter__()
                alt_pool = {"p": None}

                hstages = {}

                def load_hstage(ck):
                    t = hsb.tile([128, 16, CK], BF16, tag="hstage",
                                 name=f"hstage_{ck}")
                    # two half-DMAs: the first projection's ht 0..7 matmuls
                    # start as soon as the first half lands
                    nc.sync.dma_start(
                        out=t[:, 0:8, :],
                        in_=hsT_tiled[:, 0:8, ck * CK:(ck + 1) * CK])
                    nc.sync.dma_start(
                        out=t[:, 8:16, :],
                        in_=hsT_tiled[:, 8:16, ck * CK:(ck + 1) * CK])
                    hstages[ck] = t

                qkv_ps = {}

                def qkv_half(ck, ft, half):
                    """Project one 128-feature block for one token chunk
                    (8 of 16 contraction tiles)."""
                    if half == 0:
                        qkv_ps[(ck, ft)] = gps.tile([128, CK], F32, tag="gemm", name=f"qps_{ck}_{ft}")
                    ps = qkv_ps[(ck, ft)]
                    for ht in range(half * 8, half * 8 + 8):
                        nc.tensor.matmul(
                            ps,
                            w_qkv[:, ht, ft * 128:(ft + 1) * 128],
                            hstages[ck][:, ht, :],
                            start=(ht == 0), stop=(ht == 15),
                        )
                    if half == 1:
                        nc.vector.tensor_scalar_add(
                            out=qkvT[ft][:, ck * CK:(ck + 1) * CK], in0=ps,
                            scalar1=bqkv_sb[:, ft:ft + 1],
                        )

                def qkv_unit(ck, ft):
                    qkv_half(ck, ft, 0)
                    qkv_half(ck, ft, 1)

                def kv_setup(b):
                    """Replicate K and transpose V (+ones) for batch b."""
                    sl = slice(b * S, (b + 1) * S)
                    nc.sync.dma_start(out=k2s[b][0:64, :], in_=qkvT[2][0:64, sl])
                    nc.sync.dma_start(out=k2s[b][64:128, :], in_=qkvT[2][0:64, sl])
                    nc.sync.dma_start_transpose(
                        out=voness[b], in_=qkvT[2][64:128, sl])

                ctxTs = {}

                osbs = {}

                def oproj_q(b, qh, qq, ec, use_act=False):
                    """o_proj partial for one 128-token tile, one 512-col
                    output block."""
                    ctxT = ctxTs[(b, qh)]
                    if ec == 0:
                        osb = osb_pool.tile([128, H], BF16, tag="osb",
                                            name=f"osb_{b}_{qh}_{qq}")
                        # pre-spend the osb slot-reuse wait (out-DMA done)
                        nc.vector.memset(osb[0:1, 0:1], 0.0)
                        nc.gpsimd.memset(osb[0:1, 1:2], 0.0)
                        osbs[(b, qh, qq)] = osb
                    osb = osbs[(b, qh, qq)]
                    op = (alt_pool["p"] or gps).tile(
                        [128, 512], F32, tag="gemm",
                        name=f"op_{b}_{qh}_{qq}_{ec}")
                    for ft in range(2):
                        nc.tensor.matmul(
                            op,
                            ctxT[(ft, qq)],
                            w_o[:, ft, ec * 512:(ec + 1) * 512],
                            start=(ft == 0), stop=(ft == 1),
                        )
                    dst = osb[:, ec * 512:(ec + 1) * 512]
                    if use_act and ec in (0, 2):
                        # epilogue: ACT helps (no exps left)
                        nc.scalar.copy(out=dst, in_=op)
                    else:
                        nc.vector.tensor_copy(out=dst, in_=op)
                    tok = slice(qh * 1024 + qq * 128, qh * 1024 + (qq + 1) * 128)
                    if use_act:
                        # epilogue: flush per half so the final DMA is small
                        if ec == 1:
                            nc.sync.dma_start(out=out[b, tok, 0:1024],
                                              in_=osb[:, 0:1024])
                        elif ec == 3:
                            nc.sync.dma_start(out=out[b, tok, 1024:2048],
                                              in_=osb[:, 1024:2048])
                    elif ec == 3:
                        nc.sync.dma_start(out=out[b, tok, :], in_=osb)

                # ---- fill-unit queue ----
                # Each entry: (deadline, closure). Deadline (block, g) means
                # the unit MUST be emitted before attention block `block`
                # reaches group `g` (dependency or WAR-slot-reuse order).
                fills = []

                def emit_fills(n):
                    for _ in range(min(n, len(fills))):
                        fills.pop(0)[1]()

                def force_fills(now):
                    while fills and fills[0][0] <= now:
                        fills.pop(0)[1]()
                    # overdue entries deeper in the queue (queue is kept
                    # roughly deadline-ordered, but oproj appends can
                    # interleave): emit everything up to the last overdue one
                    # to preserve front-to-back WAR ordering.
                    while any(d <= now for d, _ in fills):
                        fills.pop(0)[1]()

                att_idx = [0]

                def attention(b, qh):
                    bi = att_idx[0]
                    att_idx[0] += 1
                    q0 = b * S + qh * 1024
                    ctxT = {(ft, j): csb.tile([128, 128], BF16,
                                              tag=f"ctxT{ft}_{j}",
                                              name=f"ctxT{ft}{j}_{b}_{qh}")
                            for ft in range(2) for j in range(8)}
                    ctxTs[(b, qh)] = ctxT
                    cnorm = [nsb.tile([128, 8, 128], BF16, tag=f"cnorm{ft}",
                                      name=f"cnorm{ft}_{b}_{qh}") for ft in range(2)]
                    for g in range(QH_PER_CORE):
                        force_fills((bi, g))
                        qt = qkvT[g // 2]
                        qp = (g % 2) * 64
                        # ctx accumulated [q, d]: two 1-bank tiles, each
                        # packing 4 q-tiles of (64 ctx feats + denom).
                        banks = [xps.tile([128, 260], F32, tag="ctx",
                                          name=f"ctx{h}_{b}_{qh}_{g}")
                                 for h in range(2)]
                        # wait-carrier: spend the bank-slot WAR wait (DVE
                        # drain release) before the real t=0 start.
                        for h in range(2):
                            nc.tensor.matmul(banks[h][0:1, 0:1],
                                             k2s[b][0:64, 0:1], k2s[b][0:64, 0:1],
                                             start=True, stop=True)
                        def scores_exp(t):
                            sc = sps.tile([128, 1024], F32, tag="scores")
                            for qc in range(2):
                                nc.tensor.matmul(
                                    sc[:, qc * 512:(qc + 1) * 512],
                                    k2s[b][qp:qp + 64, t * 128:(t + 1) * 128],
                                    qt[qp:qp + 64, q0 + qc * 512:q0 + (qc + 1) * 512],
                                    start=True, stop=True,
                                )
                            ex = asb.tile([128, 1024], BF16, tag="expT")
                            nc.scalar.activation(
                                out=ex, in_=sc, func=AF.Exp,
                                bias=mask_sb[:, b, t:t + 1], scale=SCALE,
                            )
                            return ex

                        exq = scores_exp(0)
                        for t in range(16):
                            ex = exq
                            if t < 15:
                                exq = scores_exp(t + 1)
                            for j in range(8):
                                jj = j % 4
                                nc.tensor.matmul(
                                    banks[j // 4][:, jj * 65:jj * 65 + 64],
                                    ex[:, j * 128:(j + 1) * 128],
                                    voness[b][:, t, :],
                                    start=(t == 0 and jj == 0),
                                    stop=False,
                                    skip_group_check=True,
                                )
                                nc.tensor.matmul(
                                    banks[j // 4][:, jj * 65 + 64:jj * 65 + 65],
                                    ex[:, j * 128:(j + 1) * 128],
                                    ones_sb,
                                    start=False,
                                    stop=(t == 15 and jj == 3),
                                    skip_group_check=True,
                                )
                            if t in (3, 7, 11, 15):
                                emit_fills(2)
                        # drain: gather denominator columns to SBUF, one
                        # batched reciprocal, then a mul per q-tile (bf16).
                        rcs = []
                        for h in range(2):
                            dc = dsb.tile([128, 4], F32, tag="dcol",
                                          name=f"dcol{h}_{b}_{qh}_{g}")
                            nc.vector.tensor_copy(
                                out=dc, in_=banks[h][:, 64:260:65])
                            rc = dsb.tile([128, 4], F32, tag="rcol",
                                          name=f"rcol{h}_{b}_{qh}_{g}")
                            nc.vector.reciprocal(out=rc, in_=dc)
                            rcs.append(rc)
                        for j in range(8):
                            bank = banks[j // 4]
                            jj = j % 4
                            nc.vector.tensor_scalar_mul(
                                out=cnorm[g // 2][:, j, qp:qp + 64],
                                in0=bank[:, jj * 65:jj * 65 + 64],
                                scalar1=rcs[j // 4][:, jj:jj + 1],
                            )
                        if g % 2 == 1:
                            for j in range(8):
                                nc.sync.dma_start_transpose(
                                    out=ctxT[(g // 2, j)],
                                    in_=cnorm[g // 2][:, j, :],
                                )

                # ================= schedule =================
                # Prologue: minimum work before att(0,0) can run: K/V of b0
                # (k2+vones) and Q feats 0:128 of its first 1024 tokens.
                for ck in range(4):
                    load_hstage(ck)
                # Q weight slices + o_proj weights queue behind the hstages.
                nc.sync.dma_start(out=w_qkv[:, :, 0:256], in_=wqkvT_t[:, :, 0:256])
                nc.sync.dma_start(out=w_o, in_=woT.rearrange("(t p) e -> p t e", p=128))
                for ck in range(4):
                    qkv_unit(ck, 2)
                kv_setup(0)
                qkv_unit(0, 0)
                qkv_unit(1, 0)

                # Fill inventory (queue stays deadline-ordered; hstage slot
                # reuse (bufs=4) additionally requires ck4..7 loads to come
                # after ck0..3's last reader in queue order).
                def add_qkv(dl, ck, ft):
                    for half in range(2):
                        fills.append(
                            (dl, lambda ck=ck, ft=ft, half=half:
                             qkv_half(ck, ft, half)))

                add_qkv((0, 2), 0, 1)
                add_qkv((0, 2), 1, 1)
                add_qkv((1, 0), 2, 0)
                add_qkv((1, 0), 3, 0)
                add_qkv((1, 2), 2, 1)
                add_qkv((1, 2), 3, 1)
                for ck in range(4, 8):
                    fills.append(((2, 0), lambda ck=ck: load_hstage(ck)))
                    add_qkv((2, 0), ck, 2)
                fills.append(((2, 0), lambda: kv_setup(1)))
                add_qkv((2, 0), 4, 0)
                add_qkv((2, 0), 5, 0)
                add_qkv((2, 2), 4, 1)
                add_qkv((2, 2), 5, 1)
                add_qkv((3, 0), 6, 0)
                add_qkv((3, 0), 7, 0)
                add_qkv((3, 2), 6, 1)
                add_qkv((3, 2), 7, 1)

                blocks = [(0, 0), (0, 1), (1, 0), (1, 1)]
                for bi, (b, qh) in enumerate(blocks):
                    attention(b, qh)
                    # o_proj units; must be emitted before the attention
                    # block two later reuses this block's ctxT slots.
                    for qq in range(8):
                        for ec in range(4):
                            fills.append(
                                ((bi + 2, 1),
                                 lambda b=b, qh=qh, qq=qq, ec=ec, bi=bi:
                                 oproj_q(b, qh, qq, ec, use_act=(bi == 3))))
                # drain everything except the last block's o_proj, close the
                # attention psum pools, then run the epilogue on a 4-deep ring
                force_fills((4, 99))
                _xps_cm.__exit__(None, None, None)
                _sps_cm.__exit__(None, None, None)
                _g2_cm = tc.tile_pool(name="gemm2_ps", bufs=6, space="PSUM")
                alt_pool["p"] = _g2_cm.__enter__()
                emit_fills(len(fills))
                _g2_cm.__exit__(None, None, None)

    nc.compile()
    return nc


def kernel(hidden_states, attention_mask, Wq, bq, Wk, bk, Wv, bv, Wo, bo):
    hidden_states = np.asarray(hidden_states, dtype=np.float32)
    attention_mask = np.asarray(attention_mask, dtype=np.float32)
    Wq = np.asarray(Wq, dtype=np.float32)
    Wk = np.asarray(Wk, dtype=np.float32)
    Wv = np.asarray(Wv, dtype=np.float32)
    Wo = np.asarray(Wo, dtype=np.float32)

    if "nc" not in _CACHE:
        _CACHE["nc"] = _build_program()
    nc = _CACHE["nc"]

    hsT = np.ascontiguousarray(
        hidden_states.reshape(TOK, H).T).astype(NPBF16)       # [H, B*S]
    maskp = np.ascontiguousarray(
        attention_mask.reshape(B, S // 128, 128).transpose(2, 0, 1))  # [128, B, 16]

    in_maps = []
    for c in range(N_CORES):
        wq = Wq[QF * c:QF * (c + 1)]          # [256, H]
        wk = Wk[D * c:D * (c + 1)]            # [64, H]
        wv = Wv[D * c:D * (c + 1)]            # [64, H]
        wqkvT_np = np.ascontiguousarray(
            np.concatenate([wq, wk, wv], axis=0).T).astype(NPBF16)  # [H, 384]
        woT_np = np.ascontiguousarray(
            Wo[:, QF * c:QF * (c + 1)].T).astype(NPBF16)            # [256, H]
        bqkv_np = np.ascontiguousarray(
            np.concatenate([bq[QF * c:QF * (c + 1)], bk[D * c:D * (c + 1)],
                            bv[D * c:D * (c + 1)]]).astype(np.float32)
            .reshape(3, 128).T)               # [128, 3]
        in_maps.append({
            "hsT": hsT, "wqkvT": wqkvT_np, "woT": woT_np,
            "bqkv": bqkv_np, "maskp": maskp,
        })

    _CACHE["last_in_maps"] = in_maps
    res = bass_utils.run_bass_kernel_spmd(nc, in_maps, core_ids=list(range(N_CORES)))
    acc = np.zeros((B, S, H), dtype=np.float32)
    for c in range(N_CORES):
        acc += res.results[c]["out"].astype(np.float32)
    acc += np.asarray(bo, dtype=np.float32)[None, None, :]
    return acc
